# revision 1
# baseline (speedup 1.0000x reference)
"""HeteroGAT encoder on 8 Trainium2 NeuronCores.

Strategy: edges sharded by destination-node range (12500 nodes/core). Per
relation, each destination node's incoming edges are split into power-of-two
"parts" (binary decomposition of its degree); parts are grouped into buckets
processed as [128 nodes x b slots] tiles. Per tile: indirect-DMA gather of
source rows from a replicated node table, leaky-relu+exp attention logits
(host-precomputed al values, pure input x weight functions), DVE tree-fold
weighted sums, indirect scatter-add of [sum(num*x_src) | sum(num)] partials
into per-relation DRAM accumulators. Epilogue divides by the softmax
denominator, applies the per-relation projection W_r AFTER aggregation (by
linearity), means over relations, and (layer 1) l2-normalizes + relu.
Two launches; the host exchanges the 100k x 64 layer-1 output between them.
"""
import sys
sys.path.insert(0, "/opt/trn_rl_repo")
import numpy as np
import concourse.bass as bass
import concourse.bacc as bacc
import concourse.tile as tile
from concourse import mybir
from concourse.bass_utils import run_bass_kernel_spmd

F32 = mybir.dt.float32
I32 = mybir.dt.int32
NCORES = 8
BUCKETS = [32, 16, 8, 4, 2, 1]
NEG = 0.2
N = 100000
NROWS = 100096
R = 3
IN = 128
OUT = 64

_nc_cache = {}


def _prep_edges(eis, n_nodes):
    base_sz = n_nodes // NCORES
    percore = []
    for c in range(NCORES):
        rels = []
        lo, hi = c * base_sz, (c + 1) * base_sz
        for r in range(R):
            src, dst = np.asarray(eis[r][0]), np.asarray(eis[r][1])
            sel = (dst >= lo) & (dst < hi)
            s = src[sel].astype(np.int64)
            d = (dst[sel] - lo).astype(np.int64)
            order = np.argsort(d, kind="stable")
            s, d = s[order], d[order]
            deg = np.bincount(d, minlength=base_sz)
            start = np.zeros(base_sz + 1, np.int64)
            np.cumsum(deg, out=start[1:])
            cons = np.zeros(base_sz, np.int64)
            buckets = {}
            for b in BUCKETS:
                nodes = np.where((deg & b) != 0)[0]
                if len(nodes):
                    st = start[nodes] + cons[nodes]
                    idx = st[:, None] + np.arange(b)[None, :]
                    srcs = s[idx]
                    cons[nodes] += b
                else:
                    srcs = np.zeros((0, b), np.int64)
                buckets[b] = (nodes, srcs)
            rels.append(buckets)
        percore.append(rels)
    nt = []
    for r in range(R):
        d = {}
        for b in BUCKETS:
            mx = max(len(percore[c][r][b][0]) for c in range(NCORES))
            d[b] = (mx + 127) // 128
        nt.append(d)
    out = {"nt": nt, "cores": [], "base_sz": base_sz}
    for c in range(NCORES):
        lo = c * base_sz
        gidxs, ndidxs, sidxs = [], [], []
        for r in range(R):
            gcols, ncols, scols = [], [], []
            for b in BUCKETS:
                ntb = nt[r][b]
                if ntb == 0:
                    continue
                nodes, srcs = percore[c][r][b]
                nb = len(nodes)
                npad = ntb * 128
                nodes_p = np.full(npad, -1, np.int64)
                nodes_p[:nb] = nodes
                srcs_p = np.full((npad, b), n_nodes, np.int64)
                srcs_p[:nb] = srcs
                g = srcs_p.reshape(ntb, 128, b).transpose(1, 0, 2).reshape(128, ntb * b)
                gcols.append(g)
                ndv = np.where(nodes_p >= 0, nodes_p + lo, n_nodes)
                sdv = np.where(nodes_p >= 0, nodes_p, base_sz)
                ncols.append(ndv.reshape(ntb, 128).T)
                scols.append(sdv.reshape(ntb, 128).T)
            gidxs.append(np.ascontiguousarray(np.concatenate(gcols, 1), np.int32))
            ndidxs.append(np.ascontiguousarray(np.concatenate(ncols, 1), np.int32))
            sidxs.append(np.ascontiguousarray(np.concatenate(scols, 1), np.int32))
        out["cores"].append({"gidx": gidxs, "ndidx": ndidxs, "sidx": sidxs})
    return out


def _build_layer(nt, n_nodes, nrows, inw, l2norm):
    base_sz = n_nodes // NCORES
    ntiles_out = (base_sz + 127) // 128
    arows = ((max(ntiles_out * 128, base_sz + 1) + 127) // 128) * 128
    W1 = inw + 1
    NT = [sum(nt[r][b] for b in BUCKETS) for r in range(R)]
    TOT = [sum(nt[r][b] * b for b in BUCKETS) for r in range(R)]

    nc = bacc.Bacc("TRN2", target_bir_lowering=False, debug=False,
                   enable_asserts=True, num_devices=NCORES)
    xtab = nc.dram_tensor("xtab", [nrows, inw], F32, kind="ExternalInput")
    wmat = nc.dram_tensor("wmat", [R, inw, 64], F32, kind="ExternalInput")
    bvec = nc.dram_tensor("bvec", [1, 64], F32, kind="ExternalInput")
    gidx_d = [nc.dram_tensor(f"gidx{r}", [128, TOT[r]], I32, kind="ExternalInput") for r in range(R)]
    alsrc_d = [nc.dram_tensor(f"alsrc{r}", [128, TOT[r]], F32, kind="ExternalInput") for r in range(R)]
    aldst_d = [nc.dram_tensor(f"aldst{r}", [128, NT[r]], F32, kind="ExternalInput") for r in range(R)]
    sidx_d = [nc.dram_tensor(f"sidx{r}", [128, NT[r]], I32, kind="ExternalInput") for r in range(R)]
    oslice = nc.dram_tensor("oslice", [base_sz, 64], F32, kind="ExternalOutput")
    accum = [nc.dram_tensor(f"accum{r}", [arows, W1], F32) for r in range(R)]

    def bcast(ap, n):
        return bass.AP(tensor=ap.tensor, offset=ap.offset, ap=list(ap.ap) + [[0, n]])

    with tile.TileContext(nc) as tc:
        import contextlib
        with contextlib.ExitStack() as ctx:
            singles = ctx.enter_context(tc.tile_pool(name="singles", bufs=1))
            idxp = ctx.enter_context(tc.tile_pool(name="idxp", bufs=2))
            gp = ctx.enter_context(tc.tile_pool(name="gp", bufs=3))
            ep = ctx.enter_context(tc.tile_pool(name="ep", bufs=4))
            resp = ctx.enter_context(tc.tile_pool(name="resp", bufs=4))
            psp = ctx.enter_context(tc.tile_pool(name="psp", bufs=4, space="PSUM"))
            epi = ctx.enter_context(tc.tile_pool(name="epi", bufs=3))

            zch = 2048
            ztile = singles.tile([128, zch], F32)
            nc.vector.memset(ztile[:], 0.0)
            for r in range(R):
                acc_flat = accum[r].ap().rearrange("a b -> (a b)")
                percols = arows * W1 // 128
                done = 0
                while done < percols:
                    cols = min(zch, percols - done)
                    nc.sync.dma_start(out=acc_flat[done * 128:(done + cols) * 128],
                                      in_=ztile[:, :cols])
                    done += cols

            bt = singles.tile([128, 64], F32)
            bsrc = bvec.ap()
            nc.sync.dma_start(out=bt[:], in_=bass.AP(
                tensor=bsrc.tensor, offset=bsrc.offset, ap=[[0, 128], [1, 64]]))

            for r in range(R):
                gidx_s = idxp.tile([128, TOT[r]], I32, tag="gidx")
                nc.sync.dma_start(out=gidx_s[:], in_=gidx_d[r][:, :])
                alsrc = idxp.tile([128, TOT[r]], F32, tag="alsrc")
                nc.sync.dma_start(out=alsrc[:], in_=alsrc_d[r][:, :])
                aldst = idxp.tile([128, NT[r]], F32, tag="aldst")
                nc.sync.dma_start(out=aldst[:], in_=aldst_d[r][:, :])
                sidx_s = idxp.tile([128, NT[r]], I32, tag="sidx")
                nc.sync.dma_start(out=sidx_s[:], in_=sidx_d[r][:, :])

                tglob = 0
                c0 = 0
                for b in BUCKETS:
                    for t in range(nt[r][b]):
                        g = gp.tile([128, b * inw], F32, tag="g")
                        g3 = g[:].rearrange("p (b w) -> p b w", b=b)
                        for j in range(b):
                            nc.gpsimd.indirect_dma_start(
                                out=g[:, j * inw:(j + 1) * inw], out_offset=None,
                                in_=xtab.ap(),
                                in_offset=bass.IndirectOffsetOnAxis(
                                    ap=gidx_s[:, c0 + j:c0 + j + 1], axis=0))
                        e = ep.tile([128, b], F32, tag="e")
                        u = ep.tile([128, b], F32, tag="u")
                        nc.vector.tensor_scalar(
                            out=e[:], in0=alsrc[:, c0:c0 + b],
                            scalar1=aldst[:, tglob:tglob + 1], scalar2=None,
                            op0=mybir.AluOpType.add)
                        nc.vector.tensor_scalar_mul(u[:], e[:], NEG)
                        nc.vector.tensor_tensor(out=e[:], in0=e[:], in1=u[:],
                                                op=mybir.AluOpType.max)
                        num = ep.tile([128, b], F32, tag="num")
                        nc.scalar.activation(num[:], e[:],
                                             mybir.ActivationFunctionType.Exp)
                        nc.vector.tensor_tensor(out=g3, in0=g3,
                                                in1=bcast(num[:], inw),
                                                op=mybir.AluOpType.mult)
                        d = b
                        while d > 1:
                            h = d // 2
                            nc.vector.tensor_add(g3[:, 0:h, :], g3[:, 0:h, :],
                                                 g3[:, d - h:d, :])
                            nc.vector.tensor_add(num[:, 0:h], num[:, 0:h],
                                                 num[:, d - h:d])
                            d -= h
                        res = resp.tile([128, W1], F32, tag="res")
                        nc.vector.tensor_copy(res[:, 0:inw], g3[:, 0, :])
                        nc.vector.tensor_copy(res[:, inw:W1], num[:, 0:1])
                        nc.gpsimd.indirect_dma_start(
                            out=accum[r].ap(), in_=res[:],
                            out_offset=bass.IndirectOffsetOnAxis(
                                ap=sidx_s[:, tglob:tglob + 1], axis=0),
                            in_offset=None,
                            compute_op=mybir.AluOpType.add)
                        tglob += 1
                        c0 += b

            ident = singles.tile([128, 128], F32)
            from concourse.masks import make_identity
            make_identity(nc, ident[:])
            wt = singles.tile([inw, 64 * R], F32)
            for r in range(R):
                nc.sync.dma_start(out=wt[:, 64 * r:64 * (r + 1)], in_=wmat[r, :, :])

            for t in range(ntiles_out):
                hps = psp.tile([128, 64], F32, tag="hps", space="PSUM")
                for r in range(R):
                    acc_t = epi.tile([128, W1], F32, tag="acc")
                    nc.sync.dma_start(out=acc_t[:],
                                      in_=accum[r].ap()[t * 128:(t + 1) * 128, :])
                    den = epi.tile([128, 1], F32, tag="den")
                    nc.vector.tensor_scalar_max(den[:], acc_t[:, inw:W1], 1e-30)
                    rden = epi.tile([128, 1], F32, tag="rden")
                    nc.vector.reciprocal(rden[:], den[:])
                    nc.vector.tensor_scalar(out=acc_t[:, 0:inw], in0=acc_t[:, 0:inw],
                                            scalar1=rden[:], scalar2=None,
                                            op0=mybir.AluOpType.mult)
                    tps = psp.tile([inw, 128], F32, tag="tps", space="PSUM")
                    nc.tensor.transpose(tps[:], acc_t[:, 0:inw], ident[:])
                    aggT = epi.tile([inw, 128], F32, tag="aggT")
                    nc.vector.tensor_copy(aggT[:], tps[:])
                    nc.tensor.matmul(hps[:], aggT[:], wt[:, 64 * r:64 * (r + 1)],
                                     start=(r == 0), stop=(r == R - 1))
                h = epi.tile([128, 64], F32, tag="h")
                nc.vector.tensor_scalar_mul(h[:], hps[:], 1.0 / R)
                nc.vector.tensor_add(h[:], h[:], bt[:])
                if l2norm:
                    sq = epi.tile([128, 64], F32, tag="sq")
                    nc.vector.tensor_tensor(out=sq[:], in0=h[:], in1=h[:],
                                            op=mybir.AluOpType.mult)
                    ss = epi.tile([128, 1], F32, tag="ss")
                    nc.vector.tensor_reduce(out=ss[:], in_=sq[:],
                                            axis=mybir.AxisListType.X,
                                            op=mybir.AluOpType.add)
                    nrm = epi.tile([128, 1], F32, tag="nrm")
                    nc.scalar.activation(nrm[:], ss[:],
                                         mybir.ActivationFunctionType.Sqrt)
                    nc.vector.tensor_scalar_max(nrm[:], nrm[:], 1e-12)
                    rn = epi.tile([128, 1], F32, tag="rn")
                    nc.vector.reciprocal(rn[:], nrm[:])
                    nc.vector.tensor_scalar(out=h[:], in0=h[:], scalar1=rn[:],
                                            scalar2=0.0,
                                            op0=mybir.AluOpType.mult,
                                            op1=mybir.AluOpType.max)
                rows = min(128, base_sz - t * 128)
                if rows > 0:
                    nc.sync.dma_start(out=oslice.ap()[t * 128:t * 128 + rows, :],
                                      in_=h[:rows, :])
    nc.compile()
    return nc


def _al_arrays(prep, al_pad):
    outs = []
    for c in range(NCORES):
        cc = prep["cores"][c]
        outs.append({
            "alsrc": [np.ascontiguousarray(al_pad[cc["gidx"][r], 2 * r]) for r in range(R)],
            "aldst": [np.ascontiguousarray(al_pad[cc["ndidx"][r], 2 * r + 1]) for r in range(R)],
        })
    return outs


def _run(nc, prep, alarrs, xtab, wmat, bv):
    ims = []
    for c in range(NCORES):
        m = {"xtab": xtab, "wmat": wmat, "bvec": bv}
        for r in range(R):
            m[f"gidx{r}"] = prep["cores"][c]["gidx"][r]
            m[f"alsrc{r}"] = alarrs[c]["alsrc"][r]
            m[f"aldst{r}"] = alarrs[c]["aldst"][r]
            m[f"sidx{r}"] = prep["cores"][c]["sidx"][r]
        ims.append(m)
    res = run_bass_kernel_spmd(nc, ims, core_ids=list(range(NCORES)))
    return np.concatenate([r["oslice"] for r in res.results], 0)


def kernel(x, ei0, ei1, ei2, W1, a_src1, a_dst1, b1, W2, a_src2, a_dst2, b2):
    x = np.asarray(x, np.float32)
    W1 = np.asarray(W1, np.float32); W2 = np.asarray(W2, np.float32)
    a_src1 = np.asarray(a_src1, np.float32); a_dst1 = np.asarray(a_dst1, np.float32)
    a_src2 = np.asarray(a_src2, np.float32); a_dst2 = np.asarray(a_dst2, np.float32)
    b1 = np.asarray(b1, np.float32); b2 = np.asarray(b2, np.float32)
    eis = [np.asarray(ei0), np.asarray(ei1), np.asarray(ei2)]

    prep = _prep_edges(eis, N)
    nt_key = tuple(tuple(sorted(d.items())) for d in prep["nt"])

    x_pad = np.zeros((NROWS, IN), np.float32)
    x_pad[:N] = x
    wal1 = np.stack([W1[r] @ v for r in range(R)
                     for v in (a_src1[r], a_dst1[r])], 1)  # [IN, 6]
    al1 = x_pad @ wal1
    al1[N:] = 0.0
    alarrs1 = _al_arrays(prep, al1)
    bv1 = np.ascontiguousarray(b1.mean(0)[None, :], np.float32)

    k1 = ("L1", nt_key)
    if k1 not in _nc_cache:
        _nc_cache[k1] = _build_layer(prep["nt"], N, NROWS, IN, l2norm=True)
    h1n = _run(_nc_cache[k1], prep, alarrs1, x_pad, W1, bv1)

    h_pad = np.zeros((NROWS, OUT), np.float32)
    h_pad[:N] = h1n
    wal2 = np.stack([W2[r] @ v for r in range(R)
                     for v in (a_src2[r], a_dst2[r])], 1)  # [OUT, 6]
    al2 = h_pad @ wal2
    al2[N:] = 0.0
    alarrs2 = _al_arrays(prep, al2)
    bv2 = np.ascontiguousarray(b2.mean(0)[None, :], np.float32)

    k2 = ("L2", nt_key)
    if k2 not in _nc_cache:
        _nc_cache[k2] = _build_layer(prep["nt"], N, NROWS, OUT, l2norm=False)
    out = _run(_nc_cache[k2], prep, alarrs2, h_pad, W2, bv2)
    return out.astype(np.float32)

